# revision 23
# baseline (speedup 1.0000x reference)
"""Trainium2 Bass kernel for nn_DecoderAtten (Bahdanau-attention GRU decoder step).

Sharding: batch-parallel attention/GRU (16 of 128 batch rows per core) in a
transposed features-on-partitions layout, AllGather of the per-core
[1792, 16] feature blocks, then vocab-parallel output projection
(each core computes [128, 4000] of the [128, 32000] logits).
"""

import sys

if "/opt/trn_rl_repo" not in sys.path:
    sys.path.insert(0, "/opt/trn_rl_repo")

import numpy as np

import concourse.bass as bass
import concourse.mybir as mybir
import concourse.tile as tile
from concourse import bacc
from concourse.bass_utils import run_bass_kernel_spmd
from concourse.masks import make_identity

# Problem constants (hardcoded per contract)
V, EMB, ENC2, DEC = 32000, 256, 1024, 512
B, S = 128, 100
NCORES = 8
BC = B // NCORES           # 16 batch rows per core
VC = V // NCORES           # 4000 vocab cols per core
F = DEC + ENC2 + EMB       # 1792 concat feature dim
R = BC * S                 # 1600 flattened (b, s) positions per core
P = 128
KF = F // P                # 14 feature chunks
# W_out padded with a bias row + zeros to a multiple of 128 (15 chunks):
# logits = [F.T; ones] @ [W_out; b_out; 0]
KW = KF + 1                # 15
WROWS = KW * P             # 1920
NEG = -1e10

f32 = mybir.dt.float32
f32r = mybir.dt.float32r
i32 = mybir.dt.int32

_TRACE = False
_TRACE_DIR = None
_DEBUG = False


def _build_nc():
    nc = bacc.Bacc("TRN2", target_bir_lowering=False, debug=False,
                   num_devices=NCORES)

    # ---- per-core DRAM I/O ----
    encT = nc.dram_tensor("encT", [ENC2, R], f32, kind="ExternalInput")
    hT = nc.dram_tensor("hT", [DEC, BC], f32, kind="ExternalInput")
    Wattn = nc.dram_tensor("Wattn", [(DEC + ENC2) // P, P, DEC], f32,
                           kind="ExternalInput")
    v4 = nc.dram_tensor("v4", [P, DEC // P], f32, kind="ExternalInput")
    battn4 = nc.dram_tensor("battn4", [P, DEC // P], f32, kind="ExternalInput")
    maskb = nc.dram_tensor("maskb", [BC, S], f32, kind="ExternalInput")
    tok = nc.dram_tensor("tok", [BC, 1], i32, kind="ExternalInput")
    emb = nc.dram_tensor("emb", [V, EMB], f32, kind="ExternalInput")
    WihT = nc.dram_tensor("WihT", [EMB + ENC2, 3 * DEC], f32, kind="ExternalInput")
    WhhT = nc.dram_tensor("WhhT", [DEC, 3 * DEC], f32, kind="ExternalInput")
    h_nat = nc.dram_tensor("h_nat", [BC, DEC], f32, kind="ExternalInput")
    brzf = nc.dram_tensor("brzf", [1, 2 * DEC], f32, kind="ExternalInput")
    bihnf = nc.dram_tensor("bihnf", [1, DEC], f32, kind="ExternalInput")
    bhhnf = nc.dram_tensor("bhhnf", [1, DEC], f32, kind="ExternalInput")
    NVT = 8
    NV = VC // NVT
    Wout = nc.dram_tensor("Wout", [KW, NVT, P, NV], f32, kind="ExternalInput")

    logits_out = nc.dram_tensor("logits", [B, VC], f32, kind="ExternalOutput")
    hTnew_out = nc.dram_tensor("hTnew", [DEC, BC], f32, kind="ExternalOutput")
    a_out = nc.dram_tensor("a", [BC, S], f32, kind="ExternalOutput")

    KE = ENC2 // P   # 8 enc feature chunks
    KX = (EMB + ENC2) // P  # 10 GRU-x chunks
    KH = DEC // P    # 4 hidden chunks
    NT = 4           # (b,s) tiles of 400 = 4 batch rows x 100
    NSZ = R // NT    # 400

    with tile.TileContext(nc) as tc:
        _ep_cm = tc.tile_pool(name="encp", bufs=1, side="right")
        ep = _ep_cm.__enter__()
        with (
            tc.tile_pool(name="const", bufs=1) as cp,
            tc.tile_pool(name="work", bufs=1) as wp,
            tc.tile_pool(name="dram", bufs=1, space="DRAM") as dp,
        ):
            # ---------- static loads ----------
            encT_sb = ep.tile([P, KE, R], f32r)
            Wattn_sb = cp.tile([P, (DEC + ENC2) // P, DEC], f32r)
            for _c in range(12):
                _e = nc.sync if _c % 2 == 0 else nc.gpsimd
                _e.dma_start(Wattn_sb[:, _c, :],
                             Wattn.ap()[_c].bitcast(f32r))
            for _c in range(KE):
                nc.sync.dma_start(
                    encT_sb[:, _c, :],
                    encT.ap()[_c * P:(_c + 1) * P, :].bitcast(f32r))
            hT_sb = cp.tile([P, KH, BC], f32)
            nc.sync.dma_start(
                hT_sb[:], hT.ap().rearrange("(c p) b -> p c b", p=P))
            v4_sb = cp.tile([P, DEC // P], f32r)
            nc.sync.dma_start(v4_sb[:], v4.ap().bitcast(f32r))
            battn4_sb = cp.tile([P, DEC // P], f32)
            nc.sync.dma_start(battn4_sb[:], battn4.ap())
            maskb_sb = cp.tile([BC, S], f32)
            nc.sync.dma_start(maskb_sb[:], maskb.ap())
            tok_sb = cp.tile([BC, 1], i32)
            nc.sync.dma_start(tok_sb[:], tok.ap())
            h_nat_sb = cp.tile([BC, DEC], f32)
            nc.sync.dma_start(h_nat_sb[:], h_nat.ap())
            brzf_sb = cp.tile([1, 2 * DEC], f32)
            nc.sync.dma_start(brzf_sb[:], brzf.ap())
            bihnf_sb = cp.tile([1, DEC], f32)
            nc.sync.dma_start(bihnf_sb[:], bihnf.ap())
            bhhnf_sb = cp.tile([1, DEC], f32)
            nc.sync.dma_start(bhhnf_sb[:], bhhnf.ap())
            ones16_sb = cp.tile([1, BC], f32)
            nc.gpsimd.memset(ones16_sb[:], 1.0)

            ones_sb = cp.tile([1, P], f32)
            nc.gpsimd.memset(ones_sb[:], 1.0)
            ident_sb = cp.tile([P, P], f32)
            make_identity(nc, ident_sb[:])

            with tc.tile_pool(name="ps_attn", bufs=1, space="PSUM") as psa:
                # ---------- hWhb = W_h.T @ hT + b_attn ----------
                psum_h = psa.tile([P, KH, BC], f32, tag="misc", bufs=2)
                for mc in range(KH):
                    msl = slice(mc * P, (mc + 1) * P)
                    for kc in range(KH):
                        nc.tensor.matmul(
                            psum_h[:, mc, :],
                            lhsT=Wattn_sb[:, kc, msl].bitcast(f32),
                            rhs=hT_sb[:, kc, :],
                            start=(kc == 0), stop=(kc == KH - 1))
                hWhb_sb = wp.tile([P, KH, BC], f32)
                for mc in range(KH):
                    nc.vector.tensor_scalar_add(
                        hWhb_sb[:, mc, :], psum_h[:, mc, :],
                        battn4_sb[:, mc:mc + 1])

                # embedding gather + transpose + early AllGather
                emb_sb = wp.tile([BC, EMB], f32)
                nc.gpsimd.indirect_dma_start(
                    out=emb_sb[:], out_offset=None,
                    in_=emb.ap(),
                    in_offset=bass.IndirectOffsetOnAxis(ap=tok_sb[:, :1],
                                                        axis=0))
                eT_sb = wp.tile([P, EMB // P, BC], f32)
                for c in range(EMB // P):
                    psum_t = psa.tile([P, BC], f32, tag="misc", bufs=2,
                                      name=f"psum_t{c}")
                    nc.tensor.transpose(
                        psum_t[:], emb_sb[:, c * P:(c + 1) * P],
                        ident_sb[:BC, :BC])
                    nc.vector.tensor_copy(eT_sb[:, c, :], psum_t[:])
                fcT_e = dp.tile([EMB, BC], f32)
                nc.scalar.dma_start(
                    fcT_e[:].rearrange("(c p) b -> p c b", p=P), eT_sb[:])
                fT_all_e = dp.tile([NCORES * EMB, BC], f32,
                                   addr_space="Shared")
                nc.gpsimd.collective_compute(
                    "AllGather", mybir.AluOpType.bypass,
                    replica_groups=[list(range(NCORES))],
                    ins=[fcT_e[:].opt()], outs=[fT_all_e[:].opt()])

                # ---------- energy + scores ----------
                scores_sb = wp.tile([1, R], f32)
                for nt in range(NT):
                    nsl = slice(nt * NSZ, (nt + 1) * NSZ)
                    psum_s = psa.tile([1, NSZ], f32, tag="sc", bufs=1,
                                      name=f"psum_s{nt}")
                    for mc in range(KH):
                        msl = slice(mc * P, (mc + 1) * P)
                        psum_e = psa.tile([P, NSZ], f32, tag="et", bufs=2,
                                          name=f"psum_e{nt}_{mc}")
                        for kc in range(KE):
                            nc.tensor.matmul(
                                psum_e[:],
                                lhsT=Wattn_sb[:, KH + kc, msl],
                                rhs=encT_sb[:, kc, nsl],
                                start=(kc == 0), stop=(kc == KE - 1))
                        # + hWhb (broadcast over s within each batch row)
                        elin = wp.tile([P, NSZ], f32, tag="elin", bufs=3,
                                       name=f"elin{nt}_{mc}")
                        nc.vector.tensor_tensor(
                            elin[:].rearrange("p (b s) -> p b s", s=S),
                            psum_e[:].rearrange("p (b s) -> p b s", s=S),
                            hWhb_sb[:, mc, 4 * nt:4 * nt + 4, None]
                            .to_broadcast((P, 4, S)),
                            mybir.AluOpType.add)
                        et = wp.tile([P, NSZ], f32r, tag="etan", bufs=3,
                                     name=f"et{nt}_{mc}")
                        nc.scalar.activation(
                            et[:], elin[:], mybir.ActivationFunctionType.Tanh)
                        nc.tensor.matmul(
                            psum_s[:],
                            lhsT=v4_sb[:, mc:mc + 1],
                            rhs=et[:],
                            start=(mc == 0), stop=(mc == KH - 1))
                    nc.vector.tensor_copy(scores_sb[:, nsl], psum_s[:])

                # ---------- mask + softmax in [16, 100] layout ----------
                sc2 = wp.tile([BC, S], f32)
                nc.scalar.dma_start(sc2[:], scores_sb[:])
                nc.vector.tensor_tensor(
                    sc2[:], sc2[:], maskb_sb[:], mybir.AluOpType.add)
                rmax = wp.tile([BC, 1], f32)
                nc.vector.reduce_max(rmax[:], sc2[:],
                                     axis=mybir.AxisListType.X)
                nrm = wp.tile([BC, 1], f32)
                nc.vector.tensor_scalar_mul(nrm[:], rmax[:], -1.0)
                ex2 = wp.tile([BC, S], f32)
                rsum = wp.tile([BC, 1], f32)
                nc.scalar.activation(ex2[:], sc2[:],
                                     mybir.ActivationFunctionType.Exp,
                                     bias=nrm[:], accum_out=rsum[:])
                rinv = wp.tile([BC, 1], f32)
                nc.vector.reciprocal(rinv[:], rsum[:])
                a2 = wp.tile([BC, S], f32)
                nc.vector.tensor_scalar_mul(a2[:], ex2[:], rinv[:])
                nc.scalar.dma_start(a_out.ap(), a2[:])
                aflat = wp.tile([1, R], f32)
                nc.scalar.dma_start(aflat[:], a2[:])

                # ---------- A broadcast to all partitions (K=1 matmul) ----
                A_sb = wp.tile([P, R], f32)
                for c in range(NT):
                    csl = slice(c * NSZ, (c + 1) * NSZ)
                    psum_a = psa.tile([P, NSZ], f32, tag="misc", bufs=2,
                                      name=f"psum_a{c}")
                    nc.tensor.matmul(
                        psum_a[:], lhsT=ones_sb[:],
                        rhs=aflat[:, csl],
                        start=True, stop=True)
                    nc.vector.tensor_copy(A_sb[:, csl], psum_a[:])

                # ---------- weighted context (transposed) ----------
                wT_sb = wp.tile([P, KE, BC], f32)
                for ec in range(KE):
                    prod = wp.tile([P, R], f32, tag="prod", bufs=1,
                                   name=f"prod{ec}")
                    nc.vector.tensor_tensor(
                        prod[:], encT_sb[:, ec, :].bitcast(f32), A_sb[:],
                        mybir.AluOpType.mult)
                    nc.vector.reduce_sum(
                        wT_sb[:, ec, :],
                        prod[:].rearrange("p (b s) -> p b s", s=S),
                        axis=mybir.AxisListType.X)

                # AllGather of weighted (rows DEC:DEC+ENC2 of F.T)
                fcT_w = dp.tile([ENC2, BC], f32)
                nc.scalar.dma_start(
                    fcT_w[:].rearrange("(c p) b -> p c b", p=P), wT_sb[:])
                fT_all_w = dp.tile([NCORES * ENC2, BC], f32,
                                   addr_space="Shared")
                nc.gpsimd.collective_compute(
                    "AllGather", mybir.AluOpType.bypass,
                    replica_groups=[list(range(NCORES))],
                    ins=[fcT_w[:].opt()], outs=[fT_all_w[:].opt()])
                _ep_cm.__exit__(None, None, None)

            # ---------- GRU ----------
            with tc.tile_pool(name="ps_gru", bufs=1, space="PSUM") as psg:
                # gx/gh in natural orientation [16, 1536]: lhsT = xT/hT
                # chunks [128, 16], rhs = W*T rows [128, 512-slices].
                # Banks: gxr, gxz, gxn (gh r/z accumulate into gxr/gxz), ghn.
                psum_gr = psg.tile([BC, DEC], f32, tag="gr")
                psum_gz = psg.tile([BC, DEC], f32, tag="gz")
                psum_gxn = psg.tile([BC, DEC], f32, tag="xn")
                psum_ghn = psg.tile([BC, DEC], f32, tag="hn")
                gbank = [psum_gr, psum_gz, psum_gxn]
                for kc in range(KX):
                    gw = wp.tile([P, 3 * DEC], f32, tag="gw", bufs=3,
                                 name=f"gwx{kc}")
                    nc.gpsimd.dma_start(gw[:], WihT.ap()[kc * P:(kc + 1) * P, :])
                    lhs_x = eT_sb[:, kc, :] if kc < 2 else wT_sb[:, kc - 2, :]
                    for g in range(3):
                        nc.tensor.matmul(
                            gbank[g][:],
                            lhsT=lhs_x.bitcast(f32),
                            rhs=gw[:, g * DEC:(g + 1) * DEC],
                            start=(kc == 0), stop=False,
                            skip_group_check=True)
                for kc in range(KH):
                    gw = wp.tile([P, 3 * DEC], f32, tag="gw", bufs=3,
                                 name=f"gwh{kc}")
                    nc.gpsimd.dma_start(gw[:], WhhT.ap()[kc * P:(kc + 1) * P, :])
                    for g in range(3):
                        out = [psum_gr, psum_gz, psum_ghn][g]
                        nc.tensor.matmul(
                            out[:],
                            lhsT=hT_sb[:, kc, :],
                            rhs=gw[:, g * DEC:(g + 1) * DEC],
                            start=(g == 2 and kc == 0), stop=False,
                            skip_group_check=True)
                # biases via K=1 ones matmul (broadcast down partitions)
                nc.tensor.matmul(psum_gr[:], lhsT=ones16_sb[:],
                                 rhs=brzf_sb[:, :DEC], start=False, stop=True,
                                 skip_group_check=True)
                nc.tensor.matmul(psum_gz[:], lhsT=ones16_sb[:],
                                 rhs=brzf_sb[:, DEC:], start=False, stop=True,
                                 skip_group_check=True)
                nc.tensor.matmul(psum_gxn[:], lhsT=ones16_sb[:],
                                 rhs=bihnf_sb[:], start=False, stop=True,
                                 skip_group_check=True)
                nc.tensor.matmul(psum_ghn[:], lhsT=ones16_sb[:],
                                 rhs=bhhnf_sb[:], start=False, stop=True,
                                 skip_group_check=True)

                r_sb = wp.tile([BC, DEC], f32)
                z_sb = wp.tile([BC, DEC], f32)
                n_sb = wp.tile([BC, DEC], f32)
                hnew_sb = wp.tile([BC, DEC], f32)
                nc.scalar.activation(r_sb[:], psum_gr[:],
                                     mybir.ActivationFunctionType.Sigmoid)
                nc.scalar.activation(z_sb[:], psum_gz[:],
                                     mybir.ActivationFunctionType.Sigmoid)
                t1 = wp.tile([BC, DEC], f32)
                nc.vector.tensor_tensor(t1[:], r_sb[:], psum_ghn[:],
                                        mybir.AluOpType.mult)
                t2 = wp.tile([BC, DEC], f32)
                nc.vector.tensor_tensor(t2[:], psum_gxn[:], t1[:],
                                        mybir.AluOpType.add)
                nc.scalar.activation(n_sb[:], t2[:],
                                     mybir.ActivationFunctionType.Tanh)
                t3 = wp.tile([BC, DEC], f32)
                nc.vector.tensor_tensor(t3[:], h_nat_sb[:], n_sb[:],
                                        mybir.AluOpType.subtract)
                t4 = wp.tile([BC, DEC], f32)
                nc.vector.tensor_tensor(t4[:], z_sb[:], t3[:],
                                        mybir.AluOpType.mult)
                nc.vector.tensor_tensor(hnew_sb[:], n_sb[:], t4[:],
                                        mybir.AluOpType.add)

                hnewT_sb = wp.tile([P, 4, BC], f32)
                for c in range(4):
                    psum_t2 = psg.tile([P, BC], f32, tag="tr", bufs=2,
                                       name=f"psum_th{c}")
                    nc.tensor.transpose(
                        psum_t2[:], hnew_sb[:, c * P:(c + 1) * P],
                        ident_sb[:BC, :BC])
                    nc.vector.tensor_copy(hnewT_sb[:, c, :], psum_t2[:])
                nc.sync.dma_start(
                    hTnew_out.ap().rearrange("(c p) b -> p c b", p=P),
                    hnewT_sb[:])

                # ---------- split AllGather: [w;e] early (overlaps GRU), h late --
                nc.sync.dma_start(
                    hTnew_out.ap().rearrange("(c p) b -> p c b", p=P),
                    hnewT_sb[:])
                fcT_h = dp.tile([DEC, BC], f32)
                nc.scalar.dma_start(
                    fcT_h[:].rearrange("(c p) b -> p c b", p=P), hnewT_sb[:])
                fT_all_h = dp.tile([NCORES * DEC, BC], f32,
                                   addr_space="Shared")
                nc.gpsimd.collective_compute(
                    "AllGather", mybir.AluOpType.bypass,
                    replica_groups=[list(range(NCORES))],
                    ins=[fcT_h[:].opt()], outs=[fT_all_h[:].opt()])

            # ---------- logits = [F.T; ones].T @ [W_out; b_out; 0] ----------
            # k-outer over 8 PSUM banks; h_new chunks (k=0..3) consumed last
            # so the tail AllGather overlaps the W/E-chunk matmuls.
            with tc.tile_pool(name="ps_out", bufs=1, space="PSUM") as pso:
                KWV = ENC2 // P   # 8 w chunks
                KEV = EMB // P    # 2 e chunks
                FT_w = wp.tile([P, KWV, P], f32r)
                FT_e = wp.tile([P, KEV + 1, P], f32r)
                FT_h = wp.tile([P, KH, P], f32r)
                for r in range(NCORES):
                    nc.gpsimd.dma_start(
                        FT_e[:, :KEV, r * BC:(r + 1) * BC],
                        fT_all_e[r * EMB:(r + 1) * EMB]
                        .rearrange("(k p) j -> p k j", p=P).bitcast(f32r))
                nc.gpsimd.memset(FT_e[:, KEV, :].bitcast(f32), 0.0)
                nc.gpsimd.memset(FT_e[:1, KEV, :].bitcast(f32), 1.0)
                for r in range(NCORES):
                    nc.gpsimd.dma_start(
                        FT_w[:, :, r * BC:(r + 1) * BC],
                        fT_all_w[r * ENC2:(r + 1) * ENC2]
                        .rearrange("(k p) j -> p k j", p=P).bitcast(f32r))
                for r in range(NCORES):
                    nc.gpsimd.dma_start(
                        FT_h[:, :, r * BC:(r + 1) * BC],
                        fT_all_h[r * DEC:(r + 1) * DEC]
                        .rearrange("(k p) j -> p k j", p=P).bitcast(f32r))

                def ft_chunk(k):
                    # global k: 0..3 h, 4..11 w, 12..13 e, 14 ones
                    if k < KH:
                        return FT_h[:, k, :]
                    if k < KH + KWV:
                        return FT_w[:, k - KH, :]
                    return FT_e[:, k - KH - KWV, :]

                korder = ([12, 13, 14] + list(range(KH, KH + 8))
                          + list(range(KH)))
                psum_ls = [pso.tile([P, NV], f32, tag=f"lg{n}", bufs=1,
                                    name=f"psum_l{n}") for n in range(NVT)]
                for ki, k in enumerate(korder):
                    for nw in range(4):
                        wt = wp.tile([P, 2, NV], f32r, tag="wout", bufs=9,
                                     name=f"wo{k}_{nw}")
                        nc.sync.dma_start(
                            wt[:], Wout.ap()[k, 2 * nw:2 * nw + 2]
                            .rearrange("n p v -> p n v").bitcast(f32r))
                        for h in range(2):
                            n = nw * 2 + h
                            nc.tensor.matmul(
                                psum_ls[n][:], lhsT=ft_chunk(k),
                                rhs=wt[:, h, :],
                                start=(ki == 0), stop=(ki == KW - 1),
                                skip_group_check=True)
                for n in range(NVT):
                    nsl = slice(n * NV, (n + 1) * NV)
                    lsb = wp.tile([P, NV], f32, tag="lsb", bufs=2,
                                  name=f"lsb{n}")
                    nc.vector.tensor_copy(lsb[:], psum_ls[n][:])
                    nc.sync.dma_start(logits_out.ap()[:, nsl], lsb[:])

    nc.finalize()
    return nc


_NC_CACHE = None


def _get_nc():
    global _NC_CACHE
    if _NC_CACHE is None:
        _NC_CACHE = _build_nc()
    return _NC_CACHE


def kernel(tokens, hidden, encoder_outputs, mask, emb_table, W_attn, b_attn,
           v_attn, W_ih, W_hh, b_ih, b_hh, W_out, b_out):
    tokens = np.asarray(tokens)
    hidden = np.ascontiguousarray(np.asarray(hidden, np.float32))
    enc = np.asarray(encoder_outputs, np.float32)
    mask = np.asarray(mask)
    emb_table = np.ascontiguousarray(np.asarray(emb_table, np.float32))
    W_attn = np.ascontiguousarray(np.asarray(W_attn, np.float32))
    b_attn = np.asarray(b_attn, np.float32)
    v_attn = np.asarray(v_attn, np.float32)
    W_ih = np.asarray(W_ih, np.float32)
    W_hh = np.asarray(W_hh, np.float32)
    b_ih = np.asarray(b_ih, np.float32)
    b_hh = np.asarray(b_hh, np.float32)
    W_out = np.asarray(W_out, np.float32)
    b_out = np.asarray(b_out, np.float32)

    v4 = np.ascontiguousarray(v_attn.reshape(DEC // P, P).T)
    battn4 = np.ascontiguousarray(b_attn.reshape(DEC // P, P).T)
    WihT = np.ascontiguousarray(W_ih.T)
    WhhT = np.ascontiguousarray(W_hh.T)
    bsum = b_ih + b_hh
    brzf = np.ascontiguousarray(bsum[:2 * DEC].reshape(1, 2 * DEC))
    bihnf = np.ascontiguousarray(b_ih[2 * DEC:].reshape(1, DEC))
    bhhnf = np.ascontiguousarray(b_hh[2 * DEC:].reshape(1, DEC))

    in_maps = []
    for r in range(NCORES):
        bsl = slice(r * BC, (r + 1) * BC)
        vsl = slice(r * VC, (r + 1) * VC)
        encT = np.ascontiguousarray(
            enc[:, bsl, :].transpose(2, 1, 0).reshape(ENC2, R))
        woc = np.zeros((WROWS, VC), np.float32)
        woc[:F] = W_out[:, vsl]
        woc[F] = b_out[vsl]
        woc = np.ascontiguousarray(
            woc.reshape(KW, P, 8, VC // 8).transpose(0, 2, 1, 3))
        in_maps.append({
            "encT": encT,
            "hT": np.ascontiguousarray(hidden[bsl].T),
            "Wattn": W_attn.reshape((DEC + ENC2) // P, P, DEC),
            "v4": v4,
            "battn4": battn4,
            "maskb": np.ascontiguousarray(
                np.where(np.asarray(mask[bsl]) == 0, np.float32(NEG),
                         np.float32(0.0)).astype(np.float32)),
            "tok": np.ascontiguousarray(
                tokens[bsl].astype(np.int32).reshape(BC, 1)),
            "emb": emb_table,
            "WihT": WihT,
            "WhhT": WhhT,
            "h_nat": np.ascontiguousarray(hidden[bsl]),
            "brzf": brzf,
            "bihnf": bihnf,
            "bhhnf": bhhnf,
            "Wout": woc,
        })

    nc = _get_nc()
    kwargs = {}
    if _TRACE:
        kwargs = dict(trace=True, tmpdir=_TRACE_DIR)
    res = run_bass_kernel_spmd(nc, in_maps, list(range(NCORES)), **kwargs)

    logits = np.concatenate([res.results[r]["logits"] for r in range(NCORES)],
                            axis=1)
    h_new = np.concatenate(
        [res.results[r]["hTnew"].T for r in range(NCORES)], axis=0)
    a = np.concatenate([res.results[r]["a"] for r in range(NCORES)], axis=0)
    kernel.last_exec_time_ns = res.exec_time_ns
    kernel.last_results = res.results
    return logits, h_new, a


kernel.last_exec_time_ns = None


# revision 27
# speedup vs baseline: 1.0816x; 1.0816x over previous
"""Trainium2 Bass kernel for nn_DecoderAtten (Bahdanau-attention GRU decoder step).

Sharding: batch-parallel attention/GRU (16 of 128 batch rows per core) in a
transposed features-on-partitions layout, AllGather of the per-core
[1792, 16] feature blocks, then vocab-parallel output projection
(each core computes [128, 4000] of the [128, 32000] logits).
"""

import sys

if "/opt/trn_rl_repo" not in sys.path:
    sys.path.insert(0, "/opt/trn_rl_repo")

import numpy as np

import concourse.bass as bass
import concourse.mybir as mybir
import concourse.tile as tile
from concourse import bacc
from concourse.bass_utils import run_bass_kernel_spmd
from concourse.masks import make_identity

# Problem constants (hardcoded per contract)
V, EMB, ENC2, DEC = 32000, 256, 1024, 512
B, S = 128, 100
NCORES = 8
BC = B // NCORES           # 16 batch rows per core
VC = V // NCORES           # 4000 vocab cols per core
F = DEC + ENC2 + EMB       # 1792 concat feature dim
R = BC * S                 # 1600 flattened (b, s) positions per core
P = 128
KF = F // P                # 14 feature chunks
# W_out padded with a bias row + zeros to a multiple of 128 (15 chunks):
# logits = [F.T; ones] @ [W_out; b_out; 0]
KW = KF + 1                # 15
WROWS = KW * P             # 1920
NEG = -1e10

f32 = mybir.dt.float32
f32r = mybir.dt.float32r
i32 = mybir.dt.int32

_TRACE = False
_TRACE_DIR = None
_DEBUG = False


def _build_nc():
    nc = bacc.Bacc("TRN2", target_bir_lowering=False, debug=False,
                   num_devices=NCORES)

    # ---- per-core DRAM I/O ----
    encT = nc.dram_tensor("encT", [ENC2, R], f32, kind="ExternalInput")
    hT = nc.dram_tensor("hT", [DEC, BC], f32, kind="ExternalInput")
    Wattn = nc.dram_tensor("Wattn", [(DEC + ENC2) // P, P, DEC], f32,
                           kind="ExternalInput")
    v4 = nc.dram_tensor("v4", [P, DEC // P], f32, kind="ExternalInput")
    battn4 = nc.dram_tensor("battn4", [P, DEC // P], f32, kind="ExternalInput")
    maskb = nc.dram_tensor("maskb", [BC, S], f32, kind="ExternalInput")
    tok = nc.dram_tensor("tok", [BC, 1], i32, kind="ExternalInput")
    emb = nc.dram_tensor("emb", [V, EMB], f32, kind="ExternalInput")
    WihT = nc.dram_tensor("WihT", [EMB + ENC2, 3 * DEC], f32, kind="ExternalInput")
    WhhT = nc.dram_tensor("WhhT", [DEC, 3 * DEC], f32, kind="ExternalInput")
    h_nat = nc.dram_tensor("h_nat", [BC, DEC], f32, kind="ExternalInput")
    brzf = nc.dram_tensor("brzf", [1, 2 * DEC], f32, kind="ExternalInput")
    bihnf = nc.dram_tensor("bihnf", [1, DEC], f32, kind="ExternalInput")
    bhhnf = nc.dram_tensor("bhhnf", [1, DEC], f32, kind="ExternalInput")
    NVT = 8
    NV = VC // NVT
    Wout = nc.dram_tensor("Wout", [KW, NVT, P, NV], f32, kind="ExternalInput")

    logits_out = nc.dram_tensor("logits", [B, VC], f32, kind="ExternalOutput")
    hTnew_out = nc.dram_tensor("hTnew", [DEC, BC], f32, kind="ExternalOutput")
    a_out = nc.dram_tensor("a", [BC, S], f32, kind="ExternalOutput")

    KE = ENC2 // P   # 8 enc feature chunks
    KX = (EMB + ENC2) // P  # 10 GRU-x chunks
    KH = DEC // P    # 4 hidden chunks
    NT = 4           # (b,s) tiles of 400 = 4 batch rows x 100
    NSZ = R // NT    # 400

    with tile.TileContext(nc) as tc:
        _ep_cm = tc.tile_pool(name="encp", bufs=1, side="right")
        ep = _ep_cm.__enter__()
        with (
            tc.tile_pool(name="const", bufs=1) as cp,
            tc.tile_pool(name="work", bufs=1) as wp,
            tc.tile_pool(name="dram", bufs=1, space="DRAM") as dp,
        ):
            # ---------- static loads ----------
            encT_sb = ep.tile([P, KE, R], f32r)
            Wattn_sb = cp.tile([P, (DEC + ENC2) // P, DEC], f32r)
            for _c in range(12):
                _e = nc.sync if _c % 2 == 0 else nc.gpsimd
                _e.dma_start(Wattn_sb[:, _c, :],
                             Wattn.ap()[_c].bitcast(f32r))
            for _c in range(KE):
                nc.sync.dma_start(
                    encT_sb[:, _c, :],
                    encT.ap()[_c * P:(_c + 1) * P, :].bitcast(f32r))
            hT_sb = cp.tile([P, KH, BC], f32)
            nc.sync.dma_start(
                hT_sb[:], hT.ap().rearrange("(c p) b -> p c b", p=P))
            v4_sb = cp.tile([P, DEC // P], f32r)
            nc.sync.dma_start(v4_sb[:], v4.ap().bitcast(f32r))
            battn4_sb = cp.tile([P, DEC // P], f32)
            nc.sync.dma_start(battn4_sb[:], battn4.ap())
            maskb_sb = cp.tile([BC, S], f32)
            nc.sync.dma_start(maskb_sb[:], maskb.ap())
            tok_sb = cp.tile([BC, 1], i32)
            nc.sync.dma_start(tok_sb[:], tok.ap())
            h_nat_sb = cp.tile([BC, DEC], f32)
            nc.sync.dma_start(h_nat_sb[:], h_nat.ap())
            brzf_sb = cp.tile([1, 2 * DEC], f32)
            nc.sync.dma_start(brzf_sb[:], brzf.ap())
            bihnf_sb = cp.tile([1, DEC], f32)
            nc.sync.dma_start(bihnf_sb[:], bihnf.ap())
            bhhnf_sb = cp.tile([1, DEC], f32)
            nc.sync.dma_start(bhhnf_sb[:], bhhnf.ap())
            ones16_sb = cp.tile([1, BC], f32)
            nc.gpsimd.memset(ones16_sb[:], 1.0)

            ones_sb = cp.tile([1, P], f32)
            nc.gpsimd.memset(ones_sb[:], 1.0)
            ident_sb = cp.tile([P, P], f32)
            make_identity(nc, ident_sb[:])

            with tc.tile_pool(name="ps_attn", bufs=1, space="PSUM") as psa:
                # ---------- hWhb = W_h.T @ hT + b_attn ----------
                psum_h = psa.tile([P, KH, BC], f32, tag="misc", bufs=2)
                for mc in range(KH):
                    msl = slice(mc * P, (mc + 1) * P)
                    for kc in range(KH):
                        nc.tensor.matmul(
                            psum_h[:, mc, :],
                            lhsT=Wattn_sb[:, kc, msl].bitcast(f32),
                            rhs=hT_sb[:, kc, :],
                            start=(kc == 0), stop=(kc == KH - 1))
                hWhb_sb = wp.tile([P, KH, BC], f32)
                for mc in range(KH):
                    nc.vector.tensor_scalar_add(
                        hWhb_sb[:, mc, :], psum_h[:, mc, :],
                        battn4_sb[:, mc:mc + 1])

                # embedding gather + transpose + early AllGather
                emb_sb = wp.tile([BC, EMB], f32)
                nc.gpsimd.indirect_dma_start(
                    out=emb_sb[:], out_offset=None,
                    in_=emb.ap(),
                    in_offset=bass.IndirectOffsetOnAxis(ap=tok_sb[:, :1],
                                                        axis=0))
                eT_sb = wp.tile([P, EMB // P, BC], f32)
                for c in range(EMB // P):
                    psum_t = psa.tile([P, BC], f32, tag="misc", bufs=2,
                                      name=f"psum_t{c}")
                    nc.tensor.transpose(
                        psum_t[:], emb_sb[:, c * P:(c + 1) * P],
                        ident_sb[:BC, :BC])
                    nc.vector.tensor_copy(eT_sb[:, c, :], psum_t[:])
                fcT_e = dp.tile([EMB, BC], f32)
                nc.scalar.dma_start(
                    fcT_e[:].rearrange("(c p) b -> p c b", p=P), eT_sb[:])
                fT_all_e = dp.tile([NCORES * EMB, BC], f32,
                                   addr_space="Shared")
                nc.gpsimd.collective_compute(
                    "AllGather", mybir.AluOpType.bypass,
                    replica_groups=[list(range(NCORES))],
                    ins=[fcT_e[:].opt()], outs=[fT_all_e[:].opt()])

                # ---------- energy + scores ----------
                scores_sb = wp.tile([1, R], f32)
                for nt in range(NT):
                    nsl = slice(nt * NSZ, (nt + 1) * NSZ)
                    psum_s = psa.tile([1, NSZ], f32, tag="sc", bufs=1,
                                      name=f"psum_s{nt}")
                    for mc in range(KH):
                        msl = slice(mc * P, (mc + 1) * P)
                        psum_e = psa.tile([P, NSZ], f32, tag="et", bufs=2,
                                          name=f"psum_e{nt}_{mc}")
                        for kc in range(KE):
                            nc.tensor.matmul(
                                psum_e[:],
                                lhsT=Wattn_sb[:, KH + kc, msl],
                                rhs=encT_sb[:, kc, nsl],
                                start=(kc == 0), stop=(kc == KE - 1))
                        # + hWhb (broadcast over s within each batch row)
                        elin = wp.tile([P, NSZ], f32, tag="elin", bufs=3,
                                       name=f"elin{nt}_{mc}")
                        nc.vector.tensor_tensor(
                            elin[:].rearrange("p (b s) -> p b s", s=S),
                            psum_e[:].rearrange("p (b s) -> p b s", s=S),
                            hWhb_sb[:, mc, 4 * nt:4 * nt + 4, None]
                            .to_broadcast((P, 4, S)),
                            mybir.AluOpType.add)
                        et = wp.tile([P, NSZ], f32r, tag="etan", bufs=3,
                                     name=f"et{nt}_{mc}")
                        nc.scalar.activation(
                            et[:], elin[:], mybir.ActivationFunctionType.Tanh)
                        nc.tensor.matmul(
                            psum_s[:],
                            lhsT=v4_sb[:, mc:mc + 1],
                            rhs=et[:],
                            start=(mc == 0), stop=(mc == KH - 1))
                    nc.vector.tensor_copy(scores_sb[:, nsl], psum_s[:])

                # ---------- mask + softmax in [16, 100] layout ----------
                sc2 = wp.tile([BC, S], f32)
                nc.scalar.dma_start(sc2[:], scores_sb[:])
                nc.vector.tensor_tensor(
                    sc2[:], sc2[:], maskb_sb[:], mybir.AluOpType.add)
                rmax = wp.tile([BC, 1], f32)
                nc.vector.reduce_max(rmax[:], sc2[:],
                                     axis=mybir.AxisListType.X)
                nrm = wp.tile([BC, 1], f32)
                nc.vector.tensor_scalar_mul(nrm[:], rmax[:], -1.0)
                ex2 = wp.tile([BC, S], f32)
                rsum = wp.tile([BC, 1], f32)
                nc.scalar.activation(ex2[:], sc2[:],
                                     mybir.ActivationFunctionType.Exp,
                                     bias=nrm[:], accum_out=rsum[:])
                rinv = wp.tile([BC, 1], f32)
                nc.vector.reciprocal(rinv[:], rsum[:])
                a2 = wp.tile([BC, S], f32)
                nc.vector.tensor_scalar_mul(a2[:], ex2[:], rinv[:])
                nc.scalar.dma_start(a_out.ap(), a2[:])
                aflat = wp.tile([1, R], f32)
                nc.scalar.dma_start(aflat[:], a2[:])

                # ---------- A broadcast to all partitions (K=1 matmul) ----
                A_sb = wp.tile([P, R], f32)
                for c in range(NT):
                    csl = slice(c * NSZ, (c + 1) * NSZ)
                    psum_a = psa.tile([P, NSZ], f32, tag="misc", bufs=2,
                                      name=f"psum_a{c}")
                    nc.tensor.matmul(
                        psum_a[:], lhsT=ones_sb[:],
                        rhs=aflat[:, csl],
                        start=True, stop=True)
                    nc.vector.tensor_copy(A_sb[:, csl], psum_a[:])

                # ---------- weighted context (transposed) ----------
                wT_sb = wp.tile([P, KE, BC], f32)
                for ec in range(KE):
                    prod = wp.tile([P, R], f32, tag="prod", bufs=1,
                                   name=f"prod{ec}")
                    nc.vector.tensor_tensor(
                        prod[:], encT_sb[:, ec, :].bitcast(f32), A_sb[:],
                        mybir.AluOpType.mult)
                    nc.vector.reduce_sum(
                        wT_sb[:, ec, :],
                        prod[:].rearrange("p (b s) -> p b s", s=S),
                        axis=mybir.AxisListType.X)

                # AllGather of weighted (rows DEC:DEC+ENC2 of F.T)
                fcT_w = dp.tile([ENC2, BC], f32)
                nc.scalar.dma_start(
                    fcT_w[:].rearrange("(c p) b -> p c b", p=P), wT_sb[:])
                fT_all_w = dp.tile([NCORES * ENC2, BC], f32,
                                   addr_space="Shared")
                nc.gpsimd.collective_compute(
                    "AllGather", mybir.AluOpType.bypass,
                    replica_groups=[list(range(NCORES))],
                    ins=[fcT_w[:].opt()], outs=[fT_all_w[:].opt()])
                _ep_cm.__exit__(None, None, None)

            # ---------- GRU ----------
            with tc.tile_pool(name="ps_gru", bufs=1, space="PSUM") as psg:
                # gx/gh in natural orientation [16, 1536]: lhsT = xT/hT
                # chunks [128, 16], rhs = W*T rows [128, 512-slices].
                # Banks: gxr, gxz, gxn (gh r/z accumulate into gxr/gxz), ghn.
                psum_gr = psg.tile([BC, DEC], f32, tag="gr")
                psum_gz = psg.tile([BC, DEC], f32, tag="gz")
                psum_gxn = psg.tile([BC, DEC], f32, tag="xn")
                psum_ghn = psg.tile([BC, DEC], f32, tag="hn")
                gbank = [psum_gr, psum_gz, psum_gxn]
                for kc in range(KX):
                    gw = wp.tile([P, 3 * DEC], f32, tag="gw", bufs=3,
                                 name=f"gwx{kc}")
                    nc.scalar.dma_start(gw[:], WihT.ap()[kc * P:(kc + 1) * P, :])
                    lhs_x = eT_sb[:, kc, :] if kc < 2 else wT_sb[:, kc - 2, :]
                    for g in range(3):
                        nc.tensor.matmul(
                            gbank[g][:],
                            lhsT=lhs_x.bitcast(f32),
                            rhs=gw[:, g * DEC:(g + 1) * DEC],
                            start=(kc == 0), stop=False,
                            skip_group_check=True)
                for kc in range(KH):
                    gw = wp.tile([P, 3 * DEC], f32, tag="gw", bufs=3,
                                 name=f"gwh{kc}")
                    nc.scalar.dma_start(gw[:], WhhT.ap()[kc * P:(kc + 1) * P, :])
                    for g in range(3):
                        out = [psum_gr, psum_gz, psum_ghn][g]
                        nc.tensor.matmul(
                            out[:],
                            lhsT=hT_sb[:, kc, :],
                            rhs=gw[:, g * DEC:(g + 1) * DEC],
                            start=(g == 2 and kc == 0), stop=False,
                            skip_group_check=True)
                # biases via K=1 ones matmul (broadcast down partitions)
                nc.tensor.matmul(psum_gr[:], lhsT=ones16_sb[:],
                                 rhs=brzf_sb[:, :DEC], start=False, stop=True,
                                 skip_group_check=True)
                nc.tensor.matmul(psum_gz[:], lhsT=ones16_sb[:],
                                 rhs=brzf_sb[:, DEC:], start=False, stop=True,
                                 skip_group_check=True)
                nc.tensor.matmul(psum_gxn[:], lhsT=ones16_sb[:],
                                 rhs=bihnf_sb[:], start=False, stop=True,
                                 skip_group_check=True)
                nc.tensor.matmul(psum_ghn[:], lhsT=ones16_sb[:],
                                 rhs=bhhnf_sb[:], start=False, stop=True,
                                 skip_group_check=True)

                r_sb = wp.tile([BC, DEC], f32)
                z_sb = wp.tile([BC, DEC], f32)
                n_sb = wp.tile([BC, DEC], f32)
                hnew_sb = wp.tile([BC, DEC], f32)
                nc.scalar.activation(r_sb[:], psum_gr[:],
                                     mybir.ActivationFunctionType.Sigmoid)
                nc.scalar.activation(z_sb[:], psum_gz[:],
                                     mybir.ActivationFunctionType.Sigmoid)
                t1 = wp.tile([BC, DEC], f32)
                nc.vector.tensor_tensor(t1[:], r_sb[:], psum_ghn[:],
                                        mybir.AluOpType.mult)
                t2 = wp.tile([BC, DEC], f32)
                nc.vector.tensor_tensor(t2[:], psum_gxn[:], t1[:],
                                        mybir.AluOpType.add)
                nc.scalar.activation(n_sb[:], t2[:],
                                     mybir.ActivationFunctionType.Tanh)
                t3 = wp.tile([BC, DEC], f32)
                nc.vector.tensor_tensor(t3[:], h_nat_sb[:], n_sb[:],
                                        mybir.AluOpType.subtract)
                t4 = wp.tile([BC, DEC], f32)
                nc.vector.tensor_tensor(t4[:], z_sb[:], t3[:],
                                        mybir.AluOpType.mult)
                nc.vector.tensor_tensor(hnew_sb[:], n_sb[:], t4[:],
                                        mybir.AluOpType.add)

                hnewT_sb = wp.tile([P, 4, BC], f32)
                for c in range(4):
                    psum_t2 = psg.tile([P, BC], f32, tag="tr", bufs=2,
                                       name=f"psum_th{c}")
                    nc.tensor.transpose(
                        psum_t2[:], hnew_sb[:, c * P:(c + 1) * P],
                        ident_sb[:BC, :BC])
                    nc.vector.tensor_copy(hnewT_sb[:, c, :], psum_t2[:])
                nc.sync.dma_start(
                    hTnew_out.ap().rearrange("(c p) b -> p c b", p=P),
                    hnewT_sb[:])

                # ---------- split AllGather: [w;e] early (overlaps GRU), h late --
                nc.sync.dma_start(
                    hTnew_out.ap().rearrange("(c p) b -> p c b", p=P),
                    hnewT_sb[:])
                fcT_h = dp.tile([DEC, BC], f32)
                nc.scalar.dma_start(
                    fcT_h[:].rearrange("(c p) b -> p c b", p=P), hnewT_sb[:])
                fT_all_h = dp.tile([NCORES * DEC, BC], f32,
                                   addr_space="Shared")
                nc.gpsimd.collective_compute(
                    "AllGather", mybir.AluOpType.bypass,
                    replica_groups=[list(range(NCORES))],
                    ins=[fcT_h[:].opt()], outs=[fT_all_h[:].opt()])

            # ---------- logits = [F.T; ones].T @ [W_out; b_out; 0] ----------
            # k-outer over 8 PSUM banks; h_new chunks (k=0..3) consumed last
            # so the tail AllGather overlaps the W/E-chunk matmuls.
            with tc.tile_pool(name="ps_out", bufs=1, space="PSUM") as pso:
                KWV = ENC2 // P   # 8 w chunks
                KEV = EMB // P    # 2 e chunks
                FT_w = wp.tile([P, KWV, P], f32r)
                FT_e = wp.tile([P, KEV + 1, P], f32r)
                FT_h = wp.tile([P, KH, P], f32r)
                for r in range(NCORES):
                    nc.sync.dma_start(
                        FT_e[:, :KEV, r * BC:(r + 1) * BC],
                        fT_all_e[r * EMB:(r + 1) * EMB]
                        .rearrange("(k p) j -> p k j", p=P).bitcast(f32r))
                nc.vector.memset(FT_e[:, KEV, :].bitcast(f32), 0.0)
                nc.vector.memset(FT_e[:1, KEV, :].bitcast(f32), 1.0)
                # (FT_w / FT_h loads are emitted inline below so their
                # semaphore waits don't block the W_out stream on SyncE)

                def ft_chunk(k):
                    # global k: 0..3 h, 4..11 w, 12..13 e, 14 ones
                    if k < KH:
                        return FT_h[:, k, :]
                    if k < KH + KWV:
                        return FT_w[:, k - KH, :]
                    return FT_e[:, k - KH - KWV, :]

                NVT = 8
                NV = VC // NVT  # 500
                korder = (list(range(KH, KH + 8)) + [12, 13, 14]
                          + list(range(KH)))
                psum_ls = [pso.tile([P, NV], f32, tag=f"lg{n}", bufs=1,
                                    name=f"psum_l{n}") for n in range(NVT)]
                wtiles = {}

                def load_k(k):
                    tl = []
                    for nw in range(4):
                        wt = wp.tile([P, 2, NV], f32r, tag="wout", bufs=10,
                                     name=f"wo{k}_{nw}")
                        nc.sync.dma_start(
                            wt[:],
                            Wout.ap()[k, 2 * nw:2 * nw + 2]
                            .rearrange("n p v -> p n v").bitcast(f32r))
                        tl.append(wt)
                    wtiles[k] = tl

                def mm_k(ki, k):
                    tl = wtiles.pop(k)
                    for n in range(NVT):
                        nc.tensor.matmul(
                            psum_ls[n][:], lhsT=ft_chunk(k),
                            rhs=tl[n // 2][:, n % 2, :],
                            start=(ki == 0), stop=(ki == KW - 1),
                            skip_group_check=True)

                # prefetch the first two w chunks, then block on FT_w
                load_k(4)
                load_k(5)
                for r in range(NCORES):
                    nc.sync.dma_start(
                        FT_w[:, :, r * BC:(r + 1) * BC],
                        fT_all_w[r * ENC2:(r + 1) * ENC2]
                        .rearrange("(k p) j -> p k j", p=P).bitcast(f32r))
                for ki, k in enumerate(korder):
                    if k not in wtiles:
                        load_k(k)
                    if ki + 2 < len(korder) and korder[ki + 2] not in wtiles:
                        pass
                    if k == 0:
                        # h chunks begin: FT_h loads (blocks sync until AG-h)
                        for r in range(NCORES):
                            nc.sync.dma_start(
                                FT_h[:, :, r * BC:(r + 1) * BC],
                                fT_all_h[r * DEC:(r + 1) * DEC]
                                .rearrange("(k p) j -> p k j", p=P)
                                .bitcast(f32r))
                    mm_k(ki, k)
                for n in range(NVT):
                    nsl = slice(n * NV, (n + 1) * NV)
                    lsb = wp.tile([P, NV], f32, tag="lsb", bufs=1,
                                  name=f"lsb{n}")
                    nc.vector.tensor_copy(lsb[:], psum_ls[n][:])
                    nc.sync.dma_start(logits_out.ap()[:, nsl], lsb[:])

    nc.finalize()
    return nc


_NC_CACHE = None


def _get_nc():
    global _NC_CACHE
    if _NC_CACHE is None:
        _NC_CACHE = _build_nc()
    return _NC_CACHE


def kernel(tokens, hidden, encoder_outputs, mask, emb_table, W_attn, b_attn,
           v_attn, W_ih, W_hh, b_ih, b_hh, W_out, b_out):
    tokens = np.asarray(tokens)
    hidden = np.ascontiguousarray(np.asarray(hidden, np.float32))
    enc = np.asarray(encoder_outputs, np.float32)
    mask = np.asarray(mask)
    emb_table = np.ascontiguousarray(np.asarray(emb_table, np.float32))
    W_attn = np.ascontiguousarray(np.asarray(W_attn, np.float32))
    b_attn = np.asarray(b_attn, np.float32)
    v_attn = np.asarray(v_attn, np.float32)
    W_ih = np.asarray(W_ih, np.float32)
    W_hh = np.asarray(W_hh, np.float32)
    b_ih = np.asarray(b_ih, np.float32)
    b_hh = np.asarray(b_hh, np.float32)
    W_out = np.asarray(W_out, np.float32)
    b_out = np.asarray(b_out, np.float32)

    v4 = np.ascontiguousarray(v_attn.reshape(DEC // P, P).T)
    battn4 = np.ascontiguousarray(b_attn.reshape(DEC // P, P).T)
    WihT = np.ascontiguousarray(W_ih.T)
    WhhT = np.ascontiguousarray(W_hh.T)
    bsum = b_ih + b_hh
    brzf = np.ascontiguousarray(bsum[:2 * DEC].reshape(1, 2 * DEC))
    bihnf = np.ascontiguousarray(b_ih[2 * DEC:].reshape(1, DEC))
    bhhnf = np.ascontiguousarray(b_hh[2 * DEC:].reshape(1, DEC))

    in_maps = []
    for r in range(NCORES):
        bsl = slice(r * BC, (r + 1) * BC)
        vsl = slice(r * VC, (r + 1) * VC)
        encT = np.ascontiguousarray(
            enc[:, bsl, :].transpose(2, 1, 0).reshape(ENC2, R))
        woc = np.zeros((WROWS, VC), np.float32)
        woc[:F] = W_out[:, vsl]
        woc[F] = b_out[vsl]
        woc = np.ascontiguousarray(
            woc.reshape(KW, P, 8, VC // 8).transpose(0, 2, 1, 3))
        in_maps.append({
            "encT": encT,
            "hT": np.ascontiguousarray(hidden[bsl].T),
            "Wattn": W_attn.reshape((DEC + ENC2) // P, P, DEC),
            "v4": v4,
            "battn4": battn4,
            "maskb": np.ascontiguousarray(
                np.where(np.asarray(mask[bsl]) == 0, np.float32(NEG),
                         np.float32(0.0)).astype(np.float32)),
            "tok": np.ascontiguousarray(
                tokens[bsl].astype(np.int32).reshape(BC, 1)),
            "emb": emb_table,
            "WihT": WihT,
            "WhhT": WhhT,
            "h_nat": np.ascontiguousarray(hidden[bsl]),
            "brzf": brzf,
            "bihnf": bihnf,
            "bhhnf": bhhnf,
            "Wout": woc,
        })

    nc = _get_nc()
    kwargs = {}
    if _TRACE:
        kwargs = dict(trace=True, tmpdir=_TRACE_DIR)
    res = run_bass_kernel_spmd(nc, in_maps, list(range(NCORES)), **kwargs)

    logits = np.concatenate([res.results[r]["logits"] for r in range(NCORES)],
                            axis=1)
    h_new = np.concatenate(
        [res.results[r]["hTnew"].T for r in range(NCORES)], axis=0)
    a = np.concatenate([res.results[r]["a"] for r in range(NCORES)], axis=0)
    kernel.last_exec_time_ns = res.exec_time_ns
    kernel.last_results = res.results
    return logits, h_new, a


kernel.last_exec_time_ns = None


# revision 28
# speedup vs baseline: 1.1545x; 1.0674x over previous
"""Trainium2 Bass kernel for nn_DecoderAtten (Bahdanau-attention GRU decoder step).

Sharding: batch-parallel attention/GRU (16 of 128 batch rows per core) in a
transposed features-on-partitions layout, AllGather of the per-core
[1792, 16] feature blocks, then vocab-parallel output projection
(each core computes [128, 4000] of the [128, 32000] logits).
"""

import sys

if "/opt/trn_rl_repo" not in sys.path:
    sys.path.insert(0, "/opt/trn_rl_repo")

import numpy as np

import concourse.bass as bass
import concourse.mybir as mybir
import concourse.tile as tile
from concourse import bacc
from concourse.bass_utils import run_bass_kernel_spmd
from concourse.masks import make_identity

# Problem constants (hardcoded per contract)
V, EMB, ENC2, DEC = 32000, 256, 1024, 512
B, S = 128, 100
NCORES = 8
BC = B // NCORES           # 16 batch rows per core
VC = V // NCORES           # 4000 vocab cols per core
F = DEC + ENC2 + EMB       # 1792 concat feature dim
R = BC * S                 # 1600 flattened (b, s) positions per core
P = 128
KF = F // P                # 14 feature chunks
# W_out padded with a bias row + zeros to a multiple of 128 (15 chunks):
# logits = [F.T; ones] @ [W_out; b_out; 0]
KW = KF + 1                # 15
WROWS = KW * P             # 1920
NEG = -1e10

f32 = mybir.dt.float32
f32r = mybir.dt.float32r
i32 = mybir.dt.int32

_TRACE = False
_TRACE_DIR = None
_DEBUG = False


def _build_nc():
    nc = bacc.Bacc("TRN2", target_bir_lowering=False, debug=False,
                   num_devices=NCORES)

    # ---- per-core DRAM I/O ----
    encT = nc.dram_tensor("encT", [ENC2, R], f32, kind="ExternalInput")
    hT = nc.dram_tensor("hT", [DEC, BC], f32, kind="ExternalInput")
    Wattn = nc.dram_tensor("Wattn", [(DEC + ENC2) // P, P, DEC], f32,
                           kind="ExternalInput")
    v4 = nc.dram_tensor("v4", [P, DEC // P], f32, kind="ExternalInput")
    battn4 = nc.dram_tensor("battn4", [P, DEC // P], f32, kind="ExternalInput")
    maskb = nc.dram_tensor("maskb", [BC, S], f32, kind="ExternalInput")
    tok = nc.dram_tensor("tok", [BC, 1], i32, kind="ExternalInput")
    emb = nc.dram_tensor("emb", [V, EMB], f32, kind="ExternalInput")
    WihT = nc.dram_tensor("WihT", [EMB + ENC2, 3 * DEC], f32, kind="ExternalInput")
    WhhT = nc.dram_tensor("WhhT", [DEC, 3 * DEC], f32, kind="ExternalInput")
    h_nat = nc.dram_tensor("h_nat", [BC, DEC], f32, kind="ExternalInput")
    brzf = nc.dram_tensor("brzf", [1, 2 * DEC], f32, kind="ExternalInput")
    bihnf = nc.dram_tensor("bihnf", [1, DEC], f32, kind="ExternalInput")
    bhhnf = nc.dram_tensor("bhhnf", [1, DEC], f32, kind="ExternalInput")
    NVT = 8
    NV = VC // NVT
    Wout = nc.dram_tensor("Wout", [KW, NVT, P, NV], f32, kind="ExternalInput")

    logits_out = nc.dram_tensor("logits", [B, VC], f32, kind="ExternalOutput")
    hTnew_out = nc.dram_tensor("hTnew", [DEC, BC], f32, kind="ExternalOutput")
    a_out = nc.dram_tensor("a", [BC, S], f32, kind="ExternalOutput")

    KE = ENC2 // P   # 8 enc feature chunks
    KX = (EMB + ENC2) // P  # 10 GRU-x chunks
    KH = DEC // P    # 4 hidden chunks
    NT = 4           # (b,s) tiles of 400 = 4 batch rows x 100
    NSZ = R // NT    # 400

    with tile.TileContext(nc) as tc:
        _ep_cm = tc.tile_pool(name="encp", bufs=1, side="right")
        ep = _ep_cm.__enter__()
        with (
            tc.tile_pool(name="const", bufs=1) as cp,
            tc.tile_pool(name="work", bufs=1) as wp,
            tc.tile_pool(name="dram", bufs=1, space="DRAM") as dp,
        ):
            # ---------- static loads ----------
            encT_sb = ep.tile([P, KE, R], f32r)
            Wattn_sb = cp.tile([P, (DEC + ENC2) // P, DEC], f32r)
            for _c in range(12):
                _e = nc.sync if _c % 2 == 0 else nc.gpsimd
                _e.dma_start(Wattn_sb[:, _c, :],
                             Wattn.ap()[_c].bitcast(f32r))
            for _c in range(KE):
                nc.sync.dma_start(
                    encT_sb[:, _c, :],
                    encT.ap()[_c * P:(_c + 1) * P, :].bitcast(f32r))
            hT_sb = cp.tile([P, KH, BC], f32)
            nc.sync.dma_start(
                hT_sb[:], hT.ap().rearrange("(c p) b -> p c b", p=P))
            v4_sb = cp.tile([P, DEC // P], f32r)
            nc.sync.dma_start(v4_sb[:], v4.ap().bitcast(f32r))
            battn4_sb = cp.tile([P, DEC // P], f32)
            nc.sync.dma_start(battn4_sb[:], battn4.ap())
            maskb_sb = cp.tile([BC, S], f32)
            nc.sync.dma_start(maskb_sb[:], maskb.ap())
            tok_sb = cp.tile([BC, 1], i32)
            nc.sync.dma_start(tok_sb[:], tok.ap())
            h_nat_sb = cp.tile([BC, DEC], f32)
            nc.sync.dma_start(h_nat_sb[:], h_nat.ap())
            brzf_sb = cp.tile([1, 2 * DEC], f32)
            nc.sync.dma_start(brzf_sb[:], brzf.ap())
            bihnf_sb = cp.tile([1, DEC], f32)
            nc.sync.dma_start(bihnf_sb[:], bihnf.ap())
            bhhnf_sb = cp.tile([1, DEC], f32)
            nc.sync.dma_start(bhhnf_sb[:], bhhnf.ap())
            ones16_sb = cp.tile([1, BC], f32)
            nc.gpsimd.memset(ones16_sb[:], 1.0)

            ones_sb = cp.tile([1, P], f32)
            nc.gpsimd.memset(ones_sb[:], 1.0)
            ident_sb = cp.tile([P, P], f32)
            make_identity(nc, ident_sb[:])

            with tc.tile_pool(name="ps_attn", bufs=1, space="PSUM") as psa:
                # ---------- hWhb = W_h.T @ hT + b_attn ----------
                psum_h = psa.tile([P, KH, BC], f32, tag="misc", bufs=2)
                for mc in range(KH):
                    msl = slice(mc * P, (mc + 1) * P)
                    for kc in range(KH):
                        nc.tensor.matmul(
                            psum_h[:, mc, :],
                            lhsT=Wattn_sb[:, kc, msl].bitcast(f32),
                            rhs=hT_sb[:, kc, :],
                            start=(kc == 0), stop=(kc == KH - 1))
                hWhb_sb = wp.tile([P, KH, BC], f32)
                for mc in range(KH):
                    nc.vector.tensor_scalar_add(
                        hWhb_sb[:, mc, :], psum_h[:, mc, :],
                        battn4_sb[:, mc:mc + 1])

                # embedding gather + transpose + early AllGather
                emb_sb = wp.tile([BC, EMB], f32)
                nc.gpsimd.indirect_dma_start(
                    out=emb_sb[:], out_offset=None,
                    in_=emb.ap(),
                    in_offset=bass.IndirectOffsetOnAxis(ap=tok_sb[:, :1],
                                                        axis=0))
                eT_sb = wp.tile([P, EMB // P, BC], f32)
                for c in range(EMB // P):
                    psum_t = psa.tile([P, BC], f32, tag="misc", bufs=2,
                                      name=f"psum_t{c}")
                    nc.tensor.transpose(
                        psum_t[:], emb_sb[:, c * P:(c + 1) * P],
                        ident_sb[:BC, :BC])
                    nc.vector.tensor_copy(eT_sb[:, c, :], psum_t[:])
                fcT_e = dp.tile([EMB, BC], f32)
                nc.scalar.dma_start(
                    fcT_e[:].rearrange("(c p) b -> p c b", p=P), eT_sb[:])
                fT_all_e = dp.tile([NCORES * EMB, BC], f32,
                                   addr_space="Shared")
                nc.gpsimd.collective_compute(
                    "AllGather", mybir.AluOpType.bypass,
                    replica_groups=[list(range(NCORES))],
                    ins=[fcT_e[:].opt()], outs=[fT_all_e[:].opt()])

                # ---------- energy + scores ----------
                scores_sb = wp.tile([1, R], f32)
                for nt in range(NT):
                    nsl = slice(nt * NSZ, (nt + 1) * NSZ)
                    psum_s = psa.tile([1, NSZ], f32, tag="sc", bufs=1,
                                      name=f"psum_s{nt}")
                    for mc in range(KH):
                        msl = slice(mc * P, (mc + 1) * P)
                        psum_e = psa.tile([P, NSZ], f32, tag="et", bufs=2,
                                          name=f"psum_e{nt}_{mc}")
                        for kc in range(KE):
                            nc.tensor.matmul(
                                psum_e[:],
                                lhsT=Wattn_sb[:, KH + kc, msl],
                                rhs=encT_sb[:, kc, nsl],
                                start=(kc == 0), stop=(kc == KE - 1))
                        # + hWhb (broadcast over s within each batch row)
                        elin = wp.tile([P, NSZ], f32, tag="elin", bufs=3,
                                       name=f"elin{nt}_{mc}")
                        nc.vector.tensor_tensor(
                            elin[:].rearrange("p (b s) -> p b s", s=S),
                            psum_e[:].rearrange("p (b s) -> p b s", s=S),
                            hWhb_sb[:, mc, 4 * nt:4 * nt + 4, None]
                            .to_broadcast((P, 4, S)),
                            mybir.AluOpType.add)
                        et = wp.tile([P, NSZ], f32r, tag="etan", bufs=3,
                                     name=f"et{nt}_{mc}")
                        nc.scalar.activation(
                            et[:], elin[:], mybir.ActivationFunctionType.Tanh)
                        nc.tensor.matmul(
                            psum_s[:],
                            lhsT=v4_sb[:, mc:mc + 1],
                            rhs=et[:],
                            start=(mc == 0), stop=(mc == KH - 1))
                    nc.vector.tensor_copy(scores_sb[:, nsl], psum_s[:])

                # ---------- mask + softmax in [16, 100] layout ----------
                sc2 = wp.tile([BC, S], f32)
                nc.scalar.dma_start(sc2[:], scores_sb[:])
                nc.vector.tensor_tensor(
                    sc2[:], sc2[:], maskb_sb[:], mybir.AluOpType.add)
                rmax = wp.tile([BC, 1], f32)
                nc.vector.reduce_max(rmax[:], sc2[:],
                                     axis=mybir.AxisListType.X)
                nrm = wp.tile([BC, 1], f32)
                nc.vector.tensor_scalar_mul(nrm[:], rmax[:], -1.0)
                ex2 = wp.tile([BC, S], f32)
                rsum = wp.tile([BC, 1], f32)
                nc.scalar.activation(ex2[:], sc2[:],
                                     mybir.ActivationFunctionType.Exp,
                                     bias=nrm[:], accum_out=rsum[:])
                rinv = wp.tile([BC, 1], f32)
                nc.vector.reciprocal(rinv[:], rsum[:])
                a2 = wp.tile([BC, S], f32)
                nc.vector.tensor_scalar_mul(a2[:], ex2[:], rinv[:])
                nc.scalar.dma_start(a_out.ap(), a2[:])
                aflat = wp.tile([1, R], f32)
                nc.scalar.dma_start(aflat[:], a2[:])

                # ---------- A broadcast to all partitions (K=1 matmul) ----
                A_sb = wp.tile([P, R], f32)
                for c in range(NT):
                    csl = slice(c * NSZ, (c + 1) * NSZ)
                    psum_a = psa.tile([P, NSZ], f32, tag="misc", bufs=2,
                                      name=f"psum_a{c}")
                    nc.tensor.matmul(
                        psum_a[:], lhsT=ones_sb[:],
                        rhs=aflat[:, csl],
                        start=True, stop=True)
                    nc.vector.tensor_copy(A_sb[:, csl], psum_a[:])

                # ---------- weighted context (transposed) ----------
                wT_sb = wp.tile([P, KE, BC], f32)
                for ec in range(KE):
                    prod = wp.tile([P, R], f32, tag="prod", bufs=1,
                                   name=f"prod{ec}")
                    nc.vector.tensor_tensor(
                        prod[:], encT_sb[:, ec, :].bitcast(f32), A_sb[:],
                        mybir.AluOpType.mult)
                    nc.vector.reduce_sum(
                        wT_sb[:, ec, :],
                        prod[:].rearrange("p (b s) -> p b s", s=S),
                        axis=mybir.AxisListType.X)

                # AllGather of weighted (rows DEC:DEC+ENC2 of F.T)
                fcT_w = dp.tile([ENC2, BC], f32)
                nc.scalar.dma_start(
                    fcT_w[:].rearrange("(c p) b -> p c b", p=P), wT_sb[:])
                fT_all_w = dp.tile([NCORES * ENC2, BC], f32,
                                   addr_space="Shared")
                nc.gpsimd.collective_compute(
                    "AllGather", mybir.AluOpType.bypass,
                    replica_groups=[list(range(NCORES))],
                    ins=[fcT_w[:].opt()], outs=[fT_all_w[:].opt()])
                _ep_cm.__exit__(None, None, None)

            # ---------- GRU ----------
            with tc.tile_pool(name="ps_gru", bufs=1, space="PSUM") as psg:
                # gx/gh in natural orientation [16, 1536]: lhsT = xT/hT
                # chunks [128, 16], rhs = W*T rows [128, 512-slices].
                # Banks: gxr, gxz, gxn (gh r/z accumulate into gxr/gxz), ghn.
                psum_gr = psg.tile([BC, DEC], f32, tag="gr")
                psum_gz = psg.tile([BC, DEC], f32, tag="gz")
                psum_gxn = psg.tile([BC, DEC], f32, tag="xn")
                psum_ghn = psg.tile([BC, DEC], f32, tag="hn")
                gbank = [psum_gr, psum_gz, psum_gxn]
                for kc in range(KX):
                    gw = wp.tile([P, 3 * DEC], f32, tag="gw", bufs=3,
                                 name=f"gwx{kc}")
                    nc.sync.dma_start(gw[:], WihT.ap()[kc * P:(kc + 1) * P, :])
                    lhs_x = eT_sb[:, kc, :] if kc < 2 else wT_sb[:, kc - 2, :]
                    for g in range(3):
                        nc.tensor.matmul(
                            gbank[g][:],
                            lhsT=lhs_x.bitcast(f32),
                            rhs=gw[:, g * DEC:(g + 1) * DEC],
                            start=(kc == 0), stop=False,
                            skip_group_check=True)
                for kc in range(KH):
                    gw = wp.tile([P, 3 * DEC], f32, tag="gw", bufs=3,
                                 name=f"gwh{kc}")
                    nc.sync.dma_start(gw[:], WhhT.ap()[kc * P:(kc + 1) * P, :])
                    for g in range(3):
                        out = [psum_gr, psum_gz, psum_ghn][g]
                        nc.tensor.matmul(
                            out[:],
                            lhsT=hT_sb[:, kc, :],
                            rhs=gw[:, g * DEC:(g + 1) * DEC],
                            start=(g == 2 and kc == 0), stop=False,
                            skip_group_check=True)
                # biases via K=1 ones matmul (broadcast down partitions)
                nc.tensor.matmul(psum_gr[:], lhsT=ones16_sb[:],
                                 rhs=brzf_sb[:, :DEC], start=False, stop=True,
                                 skip_group_check=True)
                nc.tensor.matmul(psum_gz[:], lhsT=ones16_sb[:],
                                 rhs=brzf_sb[:, DEC:], start=False, stop=True,
                                 skip_group_check=True)
                nc.tensor.matmul(psum_gxn[:], lhsT=ones16_sb[:],
                                 rhs=bihnf_sb[:], start=False, stop=True,
                                 skip_group_check=True)
                nc.tensor.matmul(psum_ghn[:], lhsT=ones16_sb[:],
                                 rhs=bhhnf_sb[:], start=False, stop=True,
                                 skip_group_check=True)

                r_sb = wp.tile([BC, DEC], f32)
                z_sb = wp.tile([BC, DEC], f32)
                n_sb = wp.tile([BC, DEC], f32)
                hnew_sb = wp.tile([BC, DEC], f32)
                nc.scalar.activation(r_sb[:], psum_gr[:],
                                     mybir.ActivationFunctionType.Sigmoid)
                nc.scalar.activation(z_sb[:], psum_gz[:],
                                     mybir.ActivationFunctionType.Sigmoid)
                t1 = wp.tile([BC, DEC], f32)
                nc.vector.tensor_tensor(t1[:], r_sb[:], psum_ghn[:],
                                        mybir.AluOpType.mult)
                t2 = wp.tile([BC, DEC], f32)
                nc.vector.tensor_tensor(t2[:], psum_gxn[:], t1[:],
                                        mybir.AluOpType.add)
                nc.scalar.activation(n_sb[:], t2[:],
                                     mybir.ActivationFunctionType.Tanh)
                t3 = wp.tile([BC, DEC], f32)
                nc.vector.tensor_tensor(t3[:], h_nat_sb[:], n_sb[:],
                                        mybir.AluOpType.subtract)
                t4 = wp.tile([BC, DEC], f32)
                nc.vector.tensor_tensor(t4[:], z_sb[:], t3[:],
                                        mybir.AluOpType.mult)
                nc.vector.tensor_tensor(hnew_sb[:], n_sb[:], t4[:],
                                        mybir.AluOpType.add)

                hnewT_sb = wp.tile([P, 4, BC], f32)
                for c in range(4):
                    psum_t2 = psg.tile([P, BC], f32, tag="tr", bufs=2,
                                       name=f"psum_th{c}")
                    nc.tensor.transpose(
                        psum_t2[:], hnew_sb[:, c * P:(c + 1) * P],
                        ident_sb[:BC, :BC])
                    nc.vector.tensor_copy(hnewT_sb[:, c, :], psum_t2[:])
                nc.sync.dma_start(
                    hTnew_out.ap().rearrange("(c p) b -> p c b", p=P),
                    hnewT_sb[:])

                # ---------- split AllGather: [w;e] early (overlaps GRU), h late --
                nc.sync.dma_start(
                    hTnew_out.ap().rearrange("(c p) b -> p c b", p=P),
                    hnewT_sb[:])
                fcT_h = dp.tile([DEC, BC], f32)
                nc.scalar.dma_start(
                    fcT_h[:].rearrange("(c p) b -> p c b", p=P), hnewT_sb[:])
                fT_all_h = dp.tile([NCORES * DEC, BC], f32,
                                   addr_space="Shared")
                nc.gpsimd.collective_compute(
                    "AllGather", mybir.AluOpType.bypass,
                    replica_groups=[list(range(NCORES))],
                    ins=[fcT_h[:].opt()], outs=[fT_all_h[:].opt()])

            # ---------- logits = [F.T; ones].T @ [W_out; b_out; 0] ----------
            # k-outer over 8 PSUM banks; h_new chunks (k=0..3) consumed last
            # so the tail AllGather overlaps the W/E-chunk matmuls.
            with tc.tile_pool(name="ps_out", bufs=1, space="PSUM") as pso:
                KWV = ENC2 // P   # 8 w chunks
                KEV = EMB // P    # 2 e chunks
                FT_w = wp.tile([P, KWV, P], f32r)
                FT_e = wp.tile([P, KEV + 1, P], f32r)
                FT_h = wp.tile([P, KH, P], f32r)
                for r in range(NCORES):
                    nc.sync.dma_start(
                        FT_e[:, :KEV, r * BC:(r + 1) * BC],
                        fT_all_e[r * EMB:(r + 1) * EMB]
                        .rearrange("(k p) j -> p k j", p=P).bitcast(f32r))
                nc.vector.memset(FT_e[:, KEV, :].bitcast(f32), 0.0)
                nc.vector.memset(FT_e[:1, KEV, :].bitcast(f32), 1.0)
                # (FT_w / FT_h loads are emitted inline below so their
                # semaphore waits don't block the W_out stream on SyncE)

                def ft_chunk(k):
                    # global k: 0..3 h, 4..11 w, 12..13 e, 14 ones
                    if k < KH:
                        return FT_h[:, k, :]
                    if k < KH + KWV:
                        return FT_w[:, k - KH, :]
                    return FT_e[:, k - KH - KWV, :]

                NVT = 8
                NV = VC // NVT  # 500
                korder = (list(range(KH, KH + 8)) + [12, 13, 14]
                          + list(range(KH)))
                psum_ls = [pso.tile([P, NV], f32, tag=f"lg{n}", bufs=1,
                                    name=f"psum_l{n}") for n in range(NVT)]
                wtiles = {}

                def load_k(k):
                    tl = []
                    for nw in range(4):
                        wt = wp.tile([P, 2, NV], f32r, tag="wout", bufs=10,
                                     name=f"wo{k}_{nw}")
                        eng = nc.sync if nw % 2 == 0 else nc.gpsimd
                        eng.dma_start(
                            wt[:],
                            Wout.ap()[k, 2 * nw:2 * nw + 2]
                            .rearrange("n p v -> p n v").bitcast(f32r))
                        tl.append(wt)
                    wtiles[k] = tl

                def mm_k(ki, k):
                    tl = wtiles.pop(k)
                    for n in range(NVT):
                        nc.tensor.matmul(
                            psum_ls[n][:], lhsT=ft_chunk(k),
                            rhs=tl[n // 2][:, n % 2, :],
                            start=(ki == 0), stop=(ki == KW - 1),
                            skip_group_check=True)

                # prefetch the first two w chunks, then block on FT_w
                load_k(4)
                load_k(5)
                for r in range(NCORES):
                    nc.sync.dma_start(
                        FT_w[:, :, r * BC:(r + 1) * BC],
                        fT_all_w[r * ENC2:(r + 1) * ENC2]
                        .rearrange("(k p) j -> p k j", p=P).bitcast(f32r))
                for ki, k in enumerate(korder):
                    if k not in wtiles:
                        load_k(k)
                    if ki + 2 < len(korder) and korder[ki + 2] not in wtiles:
                        pass
                    if k == 0:
                        # h chunks begin: FT_h loads (blocks sync until AG-h)
                        for r in range(NCORES):
                            nc.sync.dma_start(
                                FT_h[:, :, r * BC:(r + 1) * BC],
                                fT_all_h[r * DEC:(r + 1) * DEC]
                                .rearrange("(k p) j -> p k j", p=P)
                                .bitcast(f32r))
                    mm_k(ki, k)
                for n in range(NVT):
                    nsl = slice(n * NV, (n + 1) * NV)
                    lsb = wp.tile([P, NV], f32, tag="lsb", bufs=1,
                                  name=f"lsb{n}")
                    nc.vector.tensor_copy(lsb[:], psum_ls[n][:])
                    nc.sync.dma_start(logits_out.ap()[:, nsl], lsb[:])

    nc.finalize()
    return nc


_NC_CACHE = None


def _get_nc():
    global _NC_CACHE
    if _NC_CACHE is None:
        _NC_CACHE = _build_nc()
    return _NC_CACHE


def kernel(tokens, hidden, encoder_outputs, mask, emb_table, W_attn, b_attn,
           v_attn, W_ih, W_hh, b_ih, b_hh, W_out, b_out):
    tokens = np.asarray(tokens)
    hidden = np.ascontiguousarray(np.asarray(hidden, np.float32))
    enc = np.asarray(encoder_outputs, np.float32)
    mask = np.asarray(mask)
    emb_table = np.ascontiguousarray(np.asarray(emb_table, np.float32))
    W_attn = np.ascontiguousarray(np.asarray(W_attn, np.float32))
    b_attn = np.asarray(b_attn, np.float32)
    v_attn = np.asarray(v_attn, np.float32)
    W_ih = np.asarray(W_ih, np.float32)
    W_hh = np.asarray(W_hh, np.float32)
    b_ih = np.asarray(b_ih, np.float32)
    b_hh = np.asarray(b_hh, np.float32)
    W_out = np.asarray(W_out, np.float32)
    b_out = np.asarray(b_out, np.float32)

    v4 = np.ascontiguousarray(v_attn.reshape(DEC // P, P).T)
    battn4 = np.ascontiguousarray(b_attn.reshape(DEC // P, P).T)
    WihT = np.ascontiguousarray(W_ih.T)
    WhhT = np.ascontiguousarray(W_hh.T)
    bsum = b_ih + b_hh
    brzf = np.ascontiguousarray(bsum[:2 * DEC].reshape(1, 2 * DEC))
    bihnf = np.ascontiguousarray(b_ih[2 * DEC:].reshape(1, DEC))
    bhhnf = np.ascontiguousarray(b_hh[2 * DEC:].reshape(1, DEC))

    in_maps = []
    for r in range(NCORES):
        bsl = slice(r * BC, (r + 1) * BC)
        vsl = slice(r * VC, (r + 1) * VC)
        encT = np.ascontiguousarray(
            enc[:, bsl, :].transpose(2, 1, 0).reshape(ENC2, R))
        woc = np.zeros((WROWS, VC), np.float32)
        woc[:F] = W_out[:, vsl]
        woc[F] = b_out[vsl]
        woc = np.ascontiguousarray(
            woc.reshape(KW, P, 8, VC // 8).transpose(0, 2, 1, 3))
        in_maps.append({
            "encT": encT,
            "hT": np.ascontiguousarray(hidden[bsl].T),
            "Wattn": W_attn.reshape((DEC + ENC2) // P, P, DEC),
            "v4": v4,
            "battn4": battn4,
            "maskb": np.ascontiguousarray(
                np.where(np.asarray(mask[bsl]) == 0, np.float32(NEG),
                         np.float32(0.0)).astype(np.float32)),
            "tok": np.ascontiguousarray(
                tokens[bsl].astype(np.int32).reshape(BC, 1)),
            "emb": emb_table,
            "WihT": WihT,
            "WhhT": WhhT,
            "h_nat": np.ascontiguousarray(hidden[bsl]),
            "brzf": brzf,
            "bihnf": bihnf,
            "bhhnf": bhhnf,
            "Wout": woc,
        })

    nc = _get_nc()
    kwargs = {}
    if _TRACE:
        kwargs = dict(trace=True, tmpdir=_TRACE_DIR)
    res = run_bass_kernel_spmd(nc, in_maps, list(range(NCORES)), **kwargs)

    logits = np.concatenate([res.results[r]["logits"] for r in range(NCORES)],
                            axis=1)
    h_new = np.concatenate(
        [res.results[r]["hTnew"].T for r in range(NCORES)], axis=0)
    a = np.concatenate([res.results[r]["a"] for r in range(NCORES)], axis=0)
    kernel.last_exec_time_ns = res.exec_time_ns
    kernel.last_results = res.results
    return logits, h_new, a


kernel.last_exec_time_ns = None
